# revision 14
# baseline (speedup 1.0000x reference)
"""Trainium2 Bass kernel for the NeuralODE problem.

Math (matching reference.py):
    20 Euler steps (10 segments x 2 steps, uniform dt => step size hi = 0.05):
        z_{i+1} = z_i + hi * ( tanh(z_i @ W1 + b1 + t_i*wt) @ W2 + b2 )

Shared device-side reformulation (per core, batch shard B=64):
    - Fold hi into W2:  W2' = hi * W2, c = hi * b2.
    - Keep the "state without accumulated c":  z'_i = z_i - i*c, so
        z'_{i+1} = z'_i + tanh(z'_i @ W1 + bias_i) @ W2'
      with bias_i = b1 + t_i*wt + i*(c @ W1)   (precomputed on host).
      Final output: z_20 = z'_20 + 20*c       (added on host).
    - State kept transposed (d-major) as zT[p, 64k+b] = z'[b, 128k+p].

Variant "b" (default): bf16 weights-STATIONARY scheme.
    - Both matmuls keep a [128,128] weight block as the stationary operand and
      stream the 64-wide batch as the moving operand (bf16 => 1 cyc/row, full
      128x128 PE utilization, LDWEIGHTS hides under the previous matmul).
      Steady-state pair rate measured ~29.5 ns (LDWEIGHTS/NX-issue bound).
    - mm1 output lands hid-major (bias is a per-partition vector there), mm2's
      moving operand is the tanh output directly, and mm2 output lands d-major
      = the state layout. No PE transposes at all (orientation chain is forced:
      mm2 must emit d-major => W2 stationary => needs h-major h => W1
      stationary for mm1).
    - PSUM layout: mm1 groups 0-11 rotate over 4 banks (ph[b] holds groups
      3b..3b+2; coarse DVE bias-add + coarse tanh per bank). mm1 tail groups
      12-15 get one region in each of the four pair-banks pp[0..3] with fused
      tanh+bias each, so no two tail groups share a bank (kills the WAR stall
      on the fused-tanh chain). mm2 pair p=dm//2 then reuses pp[p] regions 0-1.
    - State tiles zt/zbf are split into 4 per-pair tiles so mm1 of step i+1
      only waits on the zbf adds it actually reads; the first two mm1 groups
      (0 and 3, different banks) are emitted k0..k5 first so 12 matmuls cover
      the boundary chain (psum drain + zbf DVE add) of the last mm2 pair.
    - Bias tile for the coarse adds is a [128, 768] fp32 running buffer:
      bias_i is linear in i, so the device does biast += delta once per step
      (on GpSimd, off the DVE critical path) instead of DMAing 3.9 MB of
      pre-broadcast bias tiles (saves ~11 us of startup HBM traffic).
    - State accumulates in fp32 (DVE add from PSUM, deferred past mm2); a bf16
      shadow (zbf = bf16(zt + f), DVE, per pair) is the mm1 moving operand
      and unblocks the next step early (numerics: ~1.4e-3 final rel err).
    - Output DMA is split per pair and fires as soon as the final inline state
      add of that pair completes.

Variant "a": fp32 batch-stationary scheme (512-wide weight streams, PE
    transposes between the two matmuls). Exact but ~4x slower.

Sharding: pure data-parallel over batch (512 -> 8 x 64); weights replicated.
"""

import numpy as np

BS, D, HID = 512, 1024, 2048
NCORES = 8
B = BS // NCORES  # 64
NSTEP = 20
KD = D // 128  # 8 k-tiles for the D contraction
KH = HID // 128  # 16 k-tiles for the HID contraction
F32 = np.float32

VARIANT = "b"  # "b" (bf16 weights-stationary) or "a" (fp32 batch-stationary)
MM_DTYPE = "float32"  # variant a: "float32" or "float32r" (f32r fails ISA check)


# --------------------------------------------------------------------------
# Variant B: bf16 weights-stationary, no transposes
# --------------------------------------------------------------------------


def _build_program_b(split_state=True, interleave=False, remap=True, reserve=False, split3=False, warmup=0, upd4=True, tail_ph=True):
    import concourse.mybir as mybir
    from concourse import bacc
    from concourse.tile import TileContext

    nc = bacc.Bacc()
    f32 = mybir.dt.float32
    bf16 = mybir.dt.bfloat16
    TANH = mybir.ActivationFunctionType.Tanh

    def _ap(x):
        return x

    # NOTE: never put per-step ops on GpSimd here — adding that engine to the
    # steady-state dependency graph measured an ~18% slowdown of every PE
    # instruction (35 ns vs 29.5 ns per LDW+MM pair).
    _BIAS_UPD_ENGINE = nc.vector.tensor_add

    zt_in = nc.dram_tensor("zt_in", [128, KD * B], f32, kind="ExternalInput")
    zbf_in = nc.dram_tensor("zbf_in", [128, KD * B], bf16, kind="ExternalInput")
    w1_d = nc.dram_tensor("w1", [128, KD * HID], bf16, kind="ExternalInput")
    w2_d = nc.dram_tensor("w2", [128, KH * D], bf16, kind="ExternalInput")
    # biases packed per-partition: biases_d[p, i*KH + m] = bias_i[m*128 + p]
    biases_d = nc.dram_tensor("biases", [128, NSTEP * KH], f32, kind="ExternalInput")
    # coarse-path bias, pre-broadcast over batch for step 0, plus per-step delta:
    # base[p, b*192 + r*64 + c] = bias_0[(3b+r)*128 + p], r<3  (h-groups 0..11)
    bias0_d = nc.dram_tensor("bias_base", [128, 768], f32, kind="ExternalInput")
    biasd_d = nc.dram_tensor("bias_delta", [128, 768], f32, kind="ExternalInput")
    zt_out = nc.dram_tensor("zt_out", [128, KD * B], f32, kind="ExternalOutput")

    with (
        TileContext(nc) as tc,
        tc.tile_pool(name="weights", bufs=1) as wpool,
        tc.tile_pool(name="state", bufs=1) as spool,
        tc.tile_pool(name="hbuf", bufs=2) as hpool,
        tc.tile_pool(name="psumh", bufs=1, space="PSUM") as ph_pool,
        tc.tile_pool(name="psumf", bufs=1, space="PSUM") as pf_pool,
    ):
        # state in 4 per-pair tiles so consumers wait only on the adds they read
        if split_state:
            zt_t = [spool.tile([128, 2 * B], f32, tag=f"zt{p}", name="zt_t") for p in range(4)]
            zbf_t = [spool.tile([128, 2 * B], bf16, tag=f"zbf{p}", name="zbf_t") for p in range(4)]
        else:
            zt_w = spool.tile([128, KD * B], f32, tag="zt")
            zbf_w = spool.tile([128, KD * B], bf16, tag="zbf")
            zt_t = [zt_w[:, 2 * p * B : 2 * (p + 1) * B] for p in range(4)]
            zbf_t = [zbf_w[:, 2 * p * B : 2 * (p + 1) * B] for p in range(4)]
        bias_sb = wpool.tile([128, NSTEP * KH], f32, tag="bias")
        # running coarse bias tile (one per ph bank) + the per-step delta
        biast = wpool.tile([128, 768], f32, tag="biast")
        biasd = wpool.tile([128, 768], f32, tag="biasd")

        # Weights live in per-group blocks (w1: one block per hm with all its
        # k-slices; w2: one block per dm with all its q-slices) so step 0's
        # hm-outer groups stream straight off the DMA arrival order.
        w1b = [
            wpool.tile([128, KD * 128], bf16, tag=f"w1_{hm}", name="w1b")
            for hm in range(KH)
        ]
        w2b = [
            wpool.tile([128, KH * 128], bf16, tag=f"w2_{dm}", name="w2b")
            for dm in range(KD)
        ]
        # DMA order = consumption order: the first matmul needs only w1b[0] +
        # zbf; coarse bias (base/delta) is not PE-critical during the DMA-bound
        # first step, so it rides between w1 and w2.
        nc.sync.dma_start(w1b[0][:, 0 : KD * 64], w1_d[:, 0 : KD * 64])
        nc.sync.dma_start(w1b[0][:, KD * 64 : KD * 128], w1_d[:, KD * 64 : KD * 128])
        for p in range(4):
            nc.sync.dma_start(_ap(zbf_t[p]), zbf_in[:, 2 * p * B : 2 * (p + 1) * B])
        nc.sync.dma_start(bias_sb[:], biases_d[:])
        for hm in range(1, KH):
            base = hm * KD * 128
            # halves: k-blocks 0-3 then 4-7, so a group can start on half 1
            nc.sync.dma_start(w1b[hm][:, 0 : KD * 64], w1_d[:, base : base + KD * 64])
            nc.sync.dma_start(
                w1b[hm][:, KD * 64 : KD * 128],
                w1_d[:, base + KD * 64 : base + KD * 128],
            )
        nc.sync.dma_start(biast[:], bias0_d[:])
        nc.sync.dma_start(biasd[:], biasd_d[:])
        for p in range(4):
            nc.sync.dma_start(_ap(zt_t[p]), zt_in[:, 2 * p * B : 2 * (p + 1) * B])
        for dm in range(KD):
            base = dm * KH * 128
            nc.sync.dma_start(w2b[dm][:, 0 : KH * 64], w2_d[:, base : base + KH * 64])
            nc.sync.dma_start(
                w2b[dm][:, KH * 64 : KH * 128],
                w2_d[:, base + KH * 64 : base + KH * 128],
            )

        # psum tiles are persistent (bufs=1 pools are the same banks every
        # step anyway); ph[b] holds mm1 groups 3b..3b+2, pair banks pp[p]
        # hold mm1 tail group 12+p then mm2 pair p.
        # region 3 of each ph bank hosts mm1 tail group 12+b (tail_ph mode),
        # so the pp banks belong to mm2 alone and mm2's first group start
        # only WARs against the previous step's (long-done) zbf/zt adds.
        ph = [
            ph_pool.tile(
                [128, (4 if tail_ph else 3) * B], f32, tag=f"ph{b}", name="ph",
                padded_shape=[128, 8 * B],
            )
            for b in range(4)
        ]
        pp = [
            pf_pool.tile(
                [128, (2 if remap else 4) * B], f32, tag=f"pp{p}", name="pp",
                padded_shape=[128, 8 * B],
            )
            for p in range(4 if remap else 2)
        ]
        if warmup:
            # HAM warmup / idle filler: dummy matmuls on the first-arriving
            # tiles keep the PE active during the DMA-paced first steps so the
            # clock gate stays at 8/8 (cold MMs run at 1.2 GHz otherwise).
            # Huge priority => the scheduler only slots them into PE-idle
            # windows it predicts (DMA waits); they write pp[3], whose first
            # real use (mm1 tail group 15) is late in step 0.
            wz = wpool.tile([128, 128], bf16, tag="warm_stat")
            nc.sync.dma_start(wz[:], zbf_in[:, 0:128])
            for wi in range(warmup):
                mm = nc.tensor.matmul(
                    pp[3][:, 0:B], wz[:], wz[:, 0:B], start=True, stop=True
                )
                mm.ins.bass_priority = 2_000_000 + wi

        for i in range(NSTEP):
            h_bf = hpool.tile([128, KH * B], bf16, tag="hbf")

            def ph_ap(hm):
                if hm >= 12:
                    if tail_ph:
                        return ph[hm - 12][:, 3 * B : 4 * B]
                    if remap:
                        return pp[hm - 12][:, 0:B]
                    e = hm - 12
                    return pp[e % 2][:, (2 + e // 2) * B : (3 + e // 2) * B]
                return ph[hm // 3][:, (hm % 3) * B : (hm % 3 + 1) * B]

            def zbf_ap(k):
                return _ap(zbf_t[k // 2])[:, (k % 2) * B : (k % 2 + 1) * B]

            def mm1_group(hm, ks):
                for k in ks:
                    mm = nc.tensor.matmul(
                        ph_ap(hm),
                        w1b[hm][:, k * 128 : k * 128 + 128],
                        zbf_ap(k),
                        start=(k == 0),
                        stop=(k == KD - 1),
                    )
                    if reserve and i > 0 and hm in (9, 10, 11) and k < 6:
                        # boundary reservation: keep these off the scheduler's
                        # hoist list so they fill the step-boundary bubble
                        # (they are the only PE work independent of the last
                        # zbf pair once everything else is hoisted)
                        mm.ins.bass_priority = 1_000_000 + i * 100 + hm * 10 + k

            def coarse(b):
                # bank b holds groups 3b..3b+2: coarse DVE bias-add then one
                # coarse tanh over [128, 192]
                nc.vector.tensor_add(
                    ph[b][:, 0 : 3 * B],
                    ph[b][:, 0 : 3 * B],
                    biast[:, b * 192 : (b + 1) * 192],
                )
                nc.scalar.activation(
                    h_bf[:, 3 * b * B : (3 * b + 3) * B], ph[b][:, 0 : 3 * B], TANH
                )

            def fused(hm):
                # tail groups: fused tanh(x + bias) per group, one short
                # activation each so the mm1->mm2 transition tail stays short
                nc.scalar.activation(
                    h_bf[:, hm * B : hm * B + B],
                    ph_ap(hm),
                    TANH,
                    bias=bias_sb[:, i * KH + hm : i * KH + hm + 1],
                )

            # Emission order: groups 0 and 3 (different banks) run k0..k5
            # first so 12 matmuls are issueable before the k6/k7 moving
            # operands (written by the last mm2 pair's zbf add) are needed —
            # this covers the step-boundary drain+DVE+sem chain.
            if interleave:
                mm1_group(0, range(0, 6))
                mm1_group(3, range(0, 6))
                mm1_group(0, [6, 7])
                mm1_group(3, [6, 7])
                mm1_group(1, range(KD))
                mm1_group(4, range(KD))
                mm1_group(2, range(KD))
                coarse(0)
                mm1_group(5, range(KD))
                coarse(1)
                for hm in (6, 7, 8):
                    mm1_group(hm, range(KD))
                coarse(2)
                for hm in (9, 10, 11):
                    mm1_group(hm, range(KD))
                coarse(3)
            else:
                for hm in range(12):
                    mm1_group(hm, range(KD))
                    if hm % 3 == 2:
                        coarse(hm // 3)
                        if hm == 11 and upd4 and i < NSTEP - 1:
                            # advance the coarse bias tile for step i+1 in four
                            # subtile adds; each waits only its coarse_b read
                            # and fits the mid-mm1 DVE idle slots, keeping the
                            # boundary DVE queue clear
                            for b4 in range(4):
                                sl = slice(b4 * 192, (b4 + 1) * 192)
                                _BIAS_UPD_ENGINE(
                                    biast[:, sl], biast[:, sl], biasd[:, sl]
                                )
            for hm in range(12, KH):
                mm1_group(hm, range(KD))
                fused(hm)

            # ---- mm2: f[dm] = sum_q W2'[q,dm]^T h[q]; d-major PSUM ----
            # pair p = dm//2 accumulates in pp[p] regions 0-1; pp[p]'s tail-
            # group region was already consumed by fused tanh 12+p, which
            # finishes p fused-tanh slots before dm=2p starts.
            def pf_ap(dm):
                if remap:
                    return pp[dm // 2][:, (dm % 2) * B : (dm % 2 + 1) * B]
                t = dm // 2
                if t < 2:
                    return pp[t][:, (dm % 2) * B : (dm % 2 + 1) * B]
                return pp[t - 2][:, (2 + dm % 2) * B : (3 + dm % 2) * B]

            def pf_pair(p):
                if remap or p < 2:
                    return pp[p][:, 0 : 2 * B]
                return pp[p - 2][:, 2 * B : 4 * B]

            for dm in range(KD):
                for q in range(KH):
                    nc.tensor.matmul(
                        pf_ap(dm),
                        w2b[dm][:, q * 128 : q * 128 + 128],
                        h_bf[:, q * B : q * B + B],
                        start=(q == 0),
                        stop=(q == KH - 1),
                    )
                if dm == 6 and i < NSTEP - 1 and split_state and split3:
                    # pair 3 bf16 shadow add is split in half: the k6 slice
                    # unblocks one mm2 group earlier, and the boundary-critical
                    # add shrinks to [128, 64]
                    nc.vector.tensor_add(
                        _ap(zbf_t[3])[:, 0:B], _ap(zt_t[3])[:, 0:B], pf_ap(6)
                    )
                if dm % 2 == 1:
                    p = dm // 2
                    if i < NSTEP - 1:
                        # bf16 shadow add (critical path: feeds next step's mm1)
                        if p == 3 and split_state and split3:
                            nc.vector.tensor_add(
                                _ap(zbf_t[3])[:, B : 2 * B],
                                _ap(zt_t[3])[:, B : 2 * B],
                                pf_ap(7),
                            )
                        else:
                            nc.vector.tensor_add(_ap(zbf_t[p]), _ap(zt_t[p]), pf_pair(p))
                    else:
                        # final step: fp32 state add inline, then ship it out
                        nc.vector.tensor_add(_ap(zt_t[p]), _ap(zt_t[p]), pf_pair(p))
                        nc.sync.dma_start(
                            zt_out[:, 2 * p * B : 2 * (p + 1) * B], _ap(zt_t[p])
                        )
            if i < NSTEP - 1:
                # fp32 state updates deferred past mm2 (the scheduler slots
                # them right behind each pair's zbf add)
                for p in range(4):
                    nc.vector.tensor_add(_ap(zt_t[p]), _ap(zt_t[p]), pf_pair(p))
                if not upd4:
                    # advance the coarse bias tile for step i+1
                    _BIAS_UPD_ENGINE(biast[:], biast[:], biasd[:])

    nc.compile()
    return nc


# --------------------------------------------------------------------------
# Variant A: fp32 batch-stationary (original baseline)
# --------------------------------------------------------------------------


def _build_program_a(mm_dtype=MM_DTYPE, repeat=1):
    import concourse.mybir as mybir
    from concourse import bacc
    from concourse.tile import TileContext

    nc = bacc.Bacc()
    f32 = mybir.dt.float32
    mmdt = getattr(mybir.dt, mm_dtype)
    TANH = mybir.ActivationFunctionType.Tanh

    zt_in = nc.dram_tensor("zt_in", [128, KD * B], mmdt, kind="ExternalInput")
    w1_d = nc.dram_tensor("w1", [128, KD * HID], mmdt, kind="ExternalInput")
    w2_d = nc.dram_tensor("w2", [128, KH * D], mmdt, kind="ExternalInput")
    biases_d = nc.dram_tensor("biases", [NSTEP, HID], mmdt, kind="ExternalInput")
    ident_d = nc.dram_tensor("ident", [128, 128], mmdt, kind="ExternalInput")
    ones_d = nc.dram_tensor("ones", [1, B], mmdt, kind="ExternalInput")
    zt_out = nc.dram_tensor("zt_out", [128, KD * B], mmdt, kind="ExternalOutput")

    with (
        TileContext(nc) as tc,
        tc.tile_pool(name="const", bufs=1) as cpool,
        tc.tile_pool(name="weights", bufs=1) as wpool,
        tc.tile_pool(name="state", bufs=1) as spool,
        tc.tile_pool(name="work", bufs=2) as hpool,
        tc.tile_pool(name="bias", bufs=2) as bpool,
        tc.tile_pool(name="psumh", bufs=2, space="PSUM") as ph_pool,
        tc.tile_pool(name="psumt", bufs=2, space="PSUM") as pt_pool,
        tc.tile_pool(name="psumf", bufs=2, space="PSUM") as pf_pool,
    ):
        ident_sb = cpool.tile([128, 128], mmdt, tag="ident")
        nc.sync.dma_start(ident_sb[:], ident_d[:])
        ones_sb = cpool.tile([1, B], mmdt, tag="ones")
        nc.sync.dma_start(ones_sb[:], ones_d[:])

        zt = spool.tile([128, KD * B], mmdt, tag="zt")
        nc.sync.dma_start(zt[:], zt_in[:])
        hT = spool.tile([128, KH * B], mmdt, tag="hT")

        w1t = []
        for k in range(KD):
            w = wpool.tile([128, HID], mmdt, tag=f"w1_{k}")
            nc.sync.dma_start(w[:], w1_d[:, k * HID : (k + 1) * HID])
            w1t.append(w)
        w2t = []
        for k in range(KH):
            w = wpool.tile([128, D], mmdt, tag=f"w2_{k}")
            nc.sync.dma_start(w[:], w2_d[:, k * D : (k + 1) * D])
            w2t.append(w)

        def scan_body(_iv=None):
            for i in range(NSTEP):
                bias_sb = bpool.tile([1, HID], mmdt, tag="bias")
                nc.sync.dma_start(bias_sb[:], biases_d[i : i + 1, :])

                phs = []
                for g in range(2):
                    ph = ph_pool.tile([128, 512], f32, tag="ph")
                    phs.append(ph)
                    for half in range(2):
                        c = 2 * g + half
                        nc.tensor.matmul(
                            ph[64 * half : 64 * half + 64, :],
                            ones_sb[:1, :],
                            bias_sb[:1, 512 * c : 512 * c + 512],
                            start=True,
                            stop=False,
                            tile_position=(0, 64 * half),
                        )
                    for k in range(KD):
                        for half in range(2):
                            c = 2 * g + half
                            nc.tensor.matmul(
                                ph[64 * half : 64 * half + 64, :],
                                zt[:, B * k : B * k + B],
                                w1t[k][:, 512 * c : 512 * c + 512],
                                start=False,
                                stop=(k == KD - 1),
                                tile_position=(0, 64 * half),
                            )

                for g in range(2):
                    h_bm = hpool.tile([128, 512], mmdt, tag="h_bm")
                    nc.scalar.activation(h_bm[:], phs[g][:], TANH)
                    pt = pt_pool.tile([128, 512], mmdt, tag="pt")
                    for u in range(4):
                        nc.tensor.matmul(
                            pt[:, 128 * u : 128 * u + 128],
                            h_bm[:, 128 * u : 128 * u + 128],
                            ident_sb[:],
                            is_transpose=True,
                            start=True,
                            stop=True,
                        )
                    nc.vector.tensor_copy(
                        hT[:, 512 * g : 512 * g + 512].rearrange(
                            "p (h u c) -> p h u c", h=2, u=4
                        ),
                        pt[:].rearrange("p (u h c) -> p h u c", u=4, h=2),
                    )

                pf = pf_pool.tile([128, 512], f32, tag="pf")
                for k in range(KH):
                    for half in range(2):
                        nc.tensor.matmul(
                            pf[64 * half : 64 * half + 64, :],
                            hT[:, B * k : B * k + B],
                            w2t[k][:, 512 * half : 512 * half + 512],
                            start=(k == 0),
                            stop=(k == KH - 1),
                            tile_position=(0, 64 * half),
                        )

                f_bm = hpool.tile([128, 512], mmdt, tag="f_bm")
                nc.vector.tensor_copy(f_bm[:], pf[:])
                pt2 = pt_pool.tile([128, 512], mmdt, tag="pt")
                for u in range(4):
                    nc.tensor.matmul(
                        pt2[:, 128 * u : 128 * u + 128],
                        f_bm[:, 128 * u : 128 * u + 128],
                        ident_sb[:],
                        is_transpose=True,
                        start=True,
                        stop=True,
                    )
                zt_v = zt[:].rearrange("p (h u c) -> p h u c", h=2, u=4)
                nc.vector.tensor_add(
                    zt_v, zt_v, pt2[:].rearrange("p (u h c) -> p h u c", u=4, h=2)
                )

        if repeat == 1:
            scan_body()
        else:
            with tc.For_i(0, repeat, 1) as _i:
                scan_body(_i)

        nc.sync.dma_start(zt_out[:], zt[:])

    nc.compile()
    return nc


# --------------------------------------------------------------------------
# Host-side packing
# --------------------------------------------------------------------------


def _pack_zT(shard):  # [B, D] -> [128, KD*B]
    return np.ascontiguousarray(
        shard.T.reshape(KD, 128, B).transpose(1, 0, 2).reshape(128, KD * B)
    )


def _unpack_zT(zt):  # [128, KD*B] -> [B, D]
    return zt.reshape(128, KD, B).transpose(1, 0, 2).reshape(D, B).T


def _host_common(z0, t, W1, b1, wt, W2, b2):
    t = np.asarray(t, F32)
    t0s, t1s = t[:-1], t[1:]
    h_seg = (t1s - t0s) / 2.0  # N_STEPS_PER_SEG = 2
    step_ts = (
        t0s[:, None] + h_seg[:, None] * np.arange(2, dtype=F32)[None, :]
    ).reshape(-1)
    step_hs = np.repeat(h_seg, 2)
    assert np.allclose(step_hs, step_hs[0]), "non-uniform Euler steps unsupported"
    scale = F32(step_hs[0])

    c = (scale * np.asarray(b2, F32)).astype(F32)  # [D]
    cW1 = (c.astype(np.float64) @ np.asarray(W1, np.float64)).astype(F32)  # [HID]
    biases = np.stack(
        [
            (np.asarray(b1, F32) + step_ts[i] * np.asarray(wt, F32) + i * cW1).astype(
                F32
            )
            for i in range(NSTEP)
        ]
    )  # [NSTEP, HID]
    return biases, scale, c


def _tile768(v):
    # [HID] -> [128, 768]: out[p, b*192 + r*64 + c] = v[(3b+r)*128 + p], r<3
    A = np.asarray(v, F32).reshape(KH, 128)[:12].reshape(4, 3, 128)
    return np.ascontiguousarray(
        np.broadcast_to(
            A.transpose(2, 0, 1)[:, :, :, None], (128, 4, 3, B)
        ).reshape(128, 768)
    ).astype(F32)


def _make_in_maps_b(z0, t, W1, b1, wt, W2, b2):
    import ml_dtypes

    bf16 = ml_dtypes.bfloat16
    z0 = np.asarray(z0, F32)
    biases, scale, c = _host_common(z0, t, W1, b1, wt, W2, b2)

    bias_cols = np.ascontiguousarray(
        biases.reshape(NSTEP, KH, 128).transpose(2, 0, 1).reshape(128, NSTEP * KH)
    )
    # coarse-path bias: step-0 values plus the (constant) per-step increment
    bias_base = _tile768(biases[0])
    bias_delta = _tile768(biases[1] - biases[0])
    # w1p[p, hm*KD*128 + k*128 + c] = W1[k*128+p, hm*128+c]
    w1p = np.ascontiguousarray(
        np.asarray(W1, F32)
        .reshape(KD, 128, KH, 128)
        .transpose(1, 2, 0, 3)
        .reshape(128, KD * HID)
    ).astype(bf16)
    # w2p[p, dm*KH*128 + q*128 + c] = W2'[q*128+p, dm*128+c]
    w2p = np.ascontiguousarray(
        (scale * np.asarray(W2, F32))
        .astype(F32)
        .reshape(KH, 128, KD, 128)
        .transpose(1, 2, 0, 3)
        .reshape(128, KH * D)
    ).astype(bf16)

    in_maps = []
    for core in range(NCORES):
        shard = z0[core * B : (core + 1) * B]
        ztp = _pack_zT(shard)
        in_maps.append(
            {
                "zt_in": ztp,
                "zbf_in": ztp.astype(bf16),
                "w1": w1p,
                "w2": w2p,
                "biases": bias_cols,
                "bias_base": bias_base,
                "bias_delta": bias_delta,
            }
        )
    return in_maps, c


def _make_in_maps_a(z0, t, W1, b1, wt, W2, b2):
    z0 = np.asarray(z0, F32)
    biases, scale, c = _host_common(z0, t, W1, b1, wt, W2, b2)
    w1p = np.ascontiguousarray(
        np.asarray(W1, F32)
        .reshape(KD, 128, HID)
        .transpose(1, 0, 2)
        .reshape(128, KD * HID)
    )
    w2p = np.ascontiguousarray(
        (scale * np.asarray(W2, F32))
        .astype(F32)
        .reshape(KH, 128, D)
        .transpose(1, 0, 2)
        .reshape(128, KH * D)
    )
    ident = np.eye(128, dtype=F32)
    ones = np.ones((1, B), F32)
    in_maps = []
    for core in range(NCORES):
        shard = z0[core * B : (core + 1) * B]
        in_maps.append(
            {
                "zt_in": _pack_zT(shard),
                "w1": w1p,
                "w2": w2p,
                "biases": biases,
                "ident": ident,
                "ones": ones,
            }
        )
    return in_maps, c


def run(z0, t, W1, b1, wt, W2, b2, trace=False, mm_dtype=MM_DTYPE, variant=VARIANT,
        **flags):
    from concourse.bass_utils import run_bass_kernel_spmd

    if variant == "b":
        in_maps, c = _make_in_maps_b(z0, t, W1, b1, wt, W2, b2)
        nc = _build_program_b(**flags)
    else:
        in_maps, c = _make_in_maps_a(z0, t, W1, b1, wt, W2, b2)
        nc = _build_program_a(mm_dtype=mm_dtype)
    res = run_bass_kernel_spmd(nc, in_maps, core_ids=list(range(NCORES)), trace=trace)

    outs = []
    for core in range(NCORES):
        z_shard = _unpack_zT(np.asarray(res.results[core]["zt_out"], F32))
        outs.append(z_shard)
    out = np.concatenate(outs, axis=0).astype(F32)
    out = out + (NSTEP * c)[None, :].astype(F32)
    return out.astype(F32), res


def kernel(z0, t, W1, b1, wt, W2, b2):
    out, _ = run(z0, t, W1, b1, wt, W2, b2, trace=False)
    return out


# revision 15
# speedup vs baseline: 1.0599x; 1.0599x over previous
"""Trainium2 Bass kernel for the NeuralODE problem.

Math (matching reference.py):
    20 Euler steps (10 segments x 2 steps, uniform dt => step size hi = 0.05):
        z_{i+1} = z_i + hi * ( tanh(z_i @ W1 + b1 + t_i*wt) @ W2 + b2 )

Shared device-side reformulation (per core, batch shard B=64):
    - Fold hi into W2:  W2' = hi * W2, c = hi * b2.
    - Keep the "state without accumulated c":  z'_i = z_i - i*c, so
        z'_{i+1} = z'_i + tanh(z'_i @ W1 + bias_i) @ W2'
      with bias_i = b1 + t_i*wt + i*(c @ W1)   (precomputed on host).
      Final output: z_20 = z'_20 + 20*c       (added on host).
    - State kept transposed (d-major) as zT[p, 64k+b] = z'[b, 128k+p].

Variant "b" (default): bf16 weights-STATIONARY scheme.
    - Both matmuls keep a [128,128] weight block as the stationary operand and
      stream the 64-wide batch as the moving operand (bf16 => 1 cyc/row, full
      128x128 PE utilization, LDWEIGHTS hides under the previous matmul).
      Steady-state pair rate measured ~29.5 ns (LDWEIGHTS/NX-issue bound).
    - mm1 output lands hid-major (bias is a per-partition vector there), mm2's
      moving operand is the tanh output directly, and mm2 output lands d-major
      = the state layout. No PE transposes at all (orientation chain is forced:
      mm2 must emit d-major => W2 stationary => needs h-major h => W1
      stationary for mm1).
    - PSUM layout: mm1 groups 0-11 rotate over 4 banks (ph[b] holds groups
      3b..3b+2; coarse DVE bias-add + coarse tanh per bank). mm1 tail groups
      12-15 get one region in each of the four pair-banks pp[0..3] with fused
      tanh+bias each, so no two tail groups share a bank (kills the WAR stall
      on the fused-tanh chain). mm2 pair p=dm//2 then reuses pp[p] regions 0-1.
    - State tiles zt/zbf are split into 4 per-pair tiles so mm1 of step i+1
      only waits on the zbf adds it actually reads; the first two mm1 groups
      (0 and 3, different banks) are emitted k0..k5 first so 12 matmuls cover
      the boundary chain (psum drain + zbf DVE add) of the last mm2 pair.
    - Bias tile for the coarse adds is a [128, 768] fp32 running buffer:
      bias_i is linear in i, so the device does biast += delta once per step
      (on GpSimd, off the DVE critical path) instead of DMAing 3.9 MB of
      pre-broadcast bias tiles (saves ~11 us of startup HBM traffic).
    - State accumulates in fp32 (DVE add from PSUM, deferred past mm2); a bf16
      shadow (zbf = bf16(zt + f), DVE, per pair) is the mm1 moving operand
      and unblocks the next step early (numerics: ~1.4e-3 final rel err).
    - Output DMA is split per pair and fires as soon as the final inline state
      add of that pair completes.

Variant "a": fp32 batch-stationary scheme (512-wide weight streams, PE
    transposes between the two matmuls). Exact but ~4x slower.

Sharding: pure data-parallel over batch (512 -> 8 x 64); weights replicated.
"""

import numpy as np

BS, D, HID = 512, 1024, 2048
NCORES = 8
B = BS // NCORES  # 64
NSTEP = 20
KD = D // 128  # 8 k-tiles for the D contraction
KH = HID // 128  # 16 k-tiles for the HID contraction
F32 = np.float32

VARIANT = "b"  # "b" (bf16 weights-stationary) or "a" (fp32 batch-stationary)
MM_DTYPE = "float32"  # variant a: "float32" or "float32r" (f32r fails ISA check)


# --------------------------------------------------------------------------
# Variant B: bf16 weights-stationary, no transposes
# --------------------------------------------------------------------------


def _build_program_b(split_state=True, interleave=False, remap=True, reserve=False, split3=False, warmup=0, upd4=True, tail_ph=True):
    import concourse.mybir as mybir
    from concourse import bacc
    from concourse.tile import TileContext

    nc = bacc.Bacc()
    f32 = mybir.dt.float32
    bf16 = mybir.dt.bfloat16
    TANH = mybir.ActivationFunctionType.Tanh

    def _ap(x):
        return x

    # NOTE: never put per-step ops on GpSimd here — adding that engine to the
    # steady-state dependency graph measured an ~18% slowdown of every PE
    # instruction (35 ns vs 29.5 ns per LDW+MM pair).
    _BIAS_UPD_ENGINE = nc.vector.tensor_add

    zt_in = nc.dram_tensor("zt_in", [128, KD * B], f32, kind="ExternalInput")
    zbf_in = nc.dram_tensor("zbf_in", [128, KD * B], bf16, kind="ExternalInput")
    w1_d = nc.dram_tensor("w1", [128, KD * HID], bf16, kind="ExternalInput")
    w2_d = nc.dram_tensor("w2", [128, KH * D], bf16, kind="ExternalInput")
    # biases packed per-partition: biases_d[p, i*KH + m] = bias_i[m*128 + p]
    biases_d = nc.dram_tensor("biases", [128, NSTEP * KH], f32, kind="ExternalInput")
    # coarse-path bias, pre-broadcast over batch for step 0, plus per-step delta:
    # base[p, b*192 + r*64 + c] = bias_0[(3b+r)*128 + p], r<3  (h-groups 0..11)
    bias0_d = nc.dram_tensor("bias_base", [128, 768], f32, kind="ExternalInput")
    biasd_d = nc.dram_tensor("bias_delta", [128, 768], f32, kind="ExternalInput")
    zt_out = nc.dram_tensor("zt_out", [128, KD * B], f32, kind="ExternalOutput")

    with (
        TileContext(nc) as tc,
        tc.tile_pool(name="weights", bufs=1) as wpool,
        tc.tile_pool(name="state", bufs=1) as spool,
        tc.tile_pool(name="hbuf", bufs=2) as hpool,
        tc.tile_pool(name="psumh", bufs=1, space="PSUM") as ph_pool,
        tc.tile_pool(name="psumf", bufs=1, space="PSUM") as pf_pool,
    ):
        # state in 4 per-pair tiles so consumers wait only on the adds they read
        if split_state:
            zt_t = [spool.tile([128, 2 * B], f32, tag=f"zt{p}", name="zt_t") for p in range(4)]
            zbf_t = [spool.tile([128, 2 * B], bf16, tag=f"zbf{p}", name="zbf_t") for p in range(4)]
        else:
            zt_w = spool.tile([128, KD * B], f32, tag="zt")
            zbf_w = spool.tile([128, KD * B], bf16, tag="zbf")
            zt_t = [zt_w[:, 2 * p * B : 2 * (p + 1) * B] for p in range(4)]
            zbf_t = [zbf_w[:, 2 * p * B : 2 * (p + 1) * B] for p in range(4)]
        bias_sb = wpool.tile([128, NSTEP * KH], f32, tag="bias")
        # running coarse bias tile (one per ph bank) + the per-step delta
        biast = wpool.tile([128, 768], f32, tag="biast")
        biasd = wpool.tile([128, 768], f32, tag="biasd")

        # Weights live in per-group blocks (w1: one block per hm with all its
        # k-slices; w2: one block per dm with all its q-slices) so step 0's
        # hm-outer groups stream straight off the DMA arrival order.
        w1b = [
            wpool.tile([128, KD * 128], bf16, tag=f"w1_{hm}", name="w1b")
            for hm in range(KH)
        ]
        w2b = [
            wpool.tile([128, KH * 128], bf16, tag=f"w2_{dm}", name="w2b")
            for dm in range(KD)
        ]
        # DMA order = consumption order: the first matmul needs only w1b[0] +
        # zbf; coarse bias (base/delta) is not PE-critical during the DMA-bound
        # first step, so it rides between w1 and w2.
        nc.sync.dma_start(w1b[0][:], w1_d[:, 0 : KD * 128])
        for p in range(4):
            nc.sync.dma_start(_ap(zbf_t[p]), zbf_in[:, 2 * p * B : 2 * (p + 1) * B])
        nc.sync.dma_start(bias_sb[:], biases_d[:])
        for hm in range(1, KH):
            nc.sync.dma_start(w1b[hm][:], w1_d[:, hm * KD * 128 : (hm + 1) * KD * 128])
        nc.sync.dma_start(biast[:], bias0_d[:])
        nc.sync.dma_start(biasd[:], biasd_d[:])
        for p in range(4):
            nc.sync.dma_start(_ap(zt_t[p]), zt_in[:, 2 * p * B : 2 * (p + 1) * B])
        for dm in range(KD):
            nc.sync.dma_start(w2b[dm][:], w2_d[:, dm * KH * 128 : (dm + 1) * KH * 128])

        # psum tiles are persistent (bufs=1 pools are the same banks every
        # step anyway); ph[b] holds mm1 groups 3b..3b+2, pair banks pp[p]
        # hold mm1 tail group 12+p then mm2 pair p.
        # region 3 of each ph bank hosts mm1 tail group 12+b (tail_ph mode),
        # so the pp banks belong to mm2 alone and mm2's first group start
        # only WARs against the previous step's (long-done) zbf/zt adds.
        ph = [
            ph_pool.tile(
                [128, (4 if tail_ph else 3) * B], f32, tag=f"ph{b}", name="ph",
                padded_shape=[128, 8 * B],
            )
            for b in range(4)
        ]
        pp = [
            pf_pool.tile(
                [128, (2 if remap else 4) * B], f32, tag=f"pp{p}", name="pp",
                padded_shape=[128, 8 * B],
            )
            for p in range(4 if remap else 2)
        ]
        if warmup:
            # HAM warmup / idle filler: dummy matmuls on the first-arriving
            # tiles keep the PE active during the DMA-paced first steps so the
            # clock gate stays at 8/8 (cold MMs run at 1.2 GHz otherwise).
            # Huge priority => the scheduler only slots them into PE-idle
            # windows it predicts (DMA waits); they write pp[3], whose first
            # real use (mm1 tail group 15) is late in step 0.
            wz = wpool.tile([128, 128], bf16, tag="warm_stat")
            nc.sync.dma_start(wz[:], zbf_in[:, 0:128])
            for wi in range(warmup):
                mm = nc.tensor.matmul(
                    pp[3][:, 0:B], wz[:], wz[:, 0:B], start=True, stop=True
                )
                mm.ins.bass_priority = 2_000_000 + wi

        for i in range(NSTEP):
            h_bf = hpool.tile([128, KH * B], bf16, tag="hbf")

            def ph_ap(hm):
                if hm >= 12:
                    if tail_ph:
                        return ph[hm - 12][:, 3 * B : 4 * B]
                    if remap:
                        return pp[hm - 12][:, 0:B]
                    e = hm - 12
                    return pp[e % 2][:, (2 + e // 2) * B : (3 + e // 2) * B]
                return ph[hm // 3][:, (hm % 3) * B : (hm % 3 + 1) * B]

            def zbf_ap(k):
                return _ap(zbf_t[k // 2])[:, (k % 2) * B : (k % 2 + 1) * B]

            def mm1_group(hm, ks):
                for k in ks:
                    mm = nc.tensor.matmul(
                        ph_ap(hm),
                        w1b[hm][:, k * 128 : k * 128 + 128],
                        zbf_ap(k),
                        start=(k == 0),
                        stop=(k == KD - 1),
                    )
                    if reserve and i > 0 and hm in (9, 10, 11) and k < 6:
                        # boundary reservation: keep these off the scheduler's
                        # hoist list so they fill the step-boundary bubble
                        # (they are the only PE work independent of the last
                        # zbf pair once everything else is hoisted)
                        mm.ins.bass_priority = 1_000_000 + i * 100 + hm * 10 + k

            def coarse(b):
                # bank b holds groups 3b..3b+2: coarse DVE bias-add then one
                # coarse tanh over [128, 192]
                nc.vector.tensor_add(
                    ph[b][:, 0 : 3 * B],
                    ph[b][:, 0 : 3 * B],
                    biast[:, b * 192 : (b + 1) * 192],
                )
                nc.scalar.activation(
                    h_bf[:, 3 * b * B : (3 * b + 3) * B], ph[b][:, 0 : 3 * B], TANH
                )

            def fused(hm):
                # tail groups: fused tanh(x + bias) per group, one short
                # activation each so the mm1->mm2 transition tail stays short
                nc.scalar.activation(
                    h_bf[:, hm * B : hm * B + B],
                    ph_ap(hm),
                    TANH,
                    bias=bias_sb[:, i * KH + hm : i * KH + hm + 1],
                )

            # Emission order: groups 0 and 3 (different banks) run k0..k5
            # first so 12 matmuls are issueable before the k6/k7 moving
            # operands (written by the last mm2 pair's zbf add) are needed —
            # this covers the step-boundary drain+DVE+sem chain.
            if interleave:
                mm1_group(0, range(0, 6))
                mm1_group(3, range(0, 6))
                mm1_group(0, [6, 7])
                mm1_group(3, [6, 7])
                mm1_group(1, range(KD))
                mm1_group(4, range(KD))
                mm1_group(2, range(KD))
                coarse(0)
                mm1_group(5, range(KD))
                coarse(1)
                for hm in (6, 7, 8):
                    mm1_group(hm, range(KD))
                coarse(2)
                for hm in (9, 10, 11):
                    mm1_group(hm, range(KD))
                coarse(3)
            else:
                for hm in range(12):
                    mm1_group(hm, range(KD))
                    if hm % 3 == 2:
                        coarse(hm // 3)
                        if hm == 11 and upd4 and i < NSTEP - 1:
                            # advance the coarse bias tile for step i+1 in four
                            # subtile adds; each waits only its coarse_b read
                            # and fits the mid-mm1 DVE idle slots, keeping the
                            # boundary DVE queue clear
                            for b4 in range(4):
                                sl = slice(b4 * 192, (b4 + 1) * 192)
                                _BIAS_UPD_ENGINE(
                                    biast[:, sl], biast[:, sl], biasd[:, sl]
                                )
            for hm in range(12, KH):
                mm1_group(hm, range(KD))
                fused(hm)

            # ---- mm2: f[dm] = sum_q W2'[q,dm]^T h[q]; d-major PSUM ----
            # pair p = dm//2 accumulates in pp[p] regions 0-1; pp[p]'s tail-
            # group region was already consumed by fused tanh 12+p, which
            # finishes p fused-tanh slots before dm=2p starts.
            def pf_ap(dm):
                if remap:
                    return pp[dm // 2][:, (dm % 2) * B : (dm % 2 + 1) * B]
                t = dm // 2
                if t < 2:
                    return pp[t][:, (dm % 2) * B : (dm % 2 + 1) * B]
                return pp[t - 2][:, (2 + dm % 2) * B : (3 + dm % 2) * B]

            def pf_pair(p):
                if remap or p < 2:
                    return pp[p][:, 0 : 2 * B]
                return pp[p - 2][:, 2 * B : 4 * B]

            for dm in range(KD):
                for q in range(KH):
                    nc.tensor.matmul(
                        pf_ap(dm),
                        w2b[dm][:, q * 128 : q * 128 + 128],
                        h_bf[:, q * B : q * B + B],
                        start=(q == 0),
                        stop=(q == KH - 1),
                    )
                if dm == 6 and i < NSTEP - 1 and split_state and split3:
                    # pair 3 bf16 shadow add is split in half: the k6 slice
                    # unblocks one mm2 group earlier, and the boundary-critical
                    # add shrinks to [128, 64]
                    nc.vector.tensor_add(
                        _ap(zbf_t[3])[:, 0:B], _ap(zt_t[3])[:, 0:B], pf_ap(6)
                    )
                if dm % 2 == 1:
                    p = dm // 2
                    if i < NSTEP - 1:
                        # bf16 shadow add (critical path: feeds next step's mm1)
                        if p == 3 and split_state and split3:
                            nc.vector.tensor_add(
                                _ap(zbf_t[3])[:, B : 2 * B],
                                _ap(zt_t[3])[:, B : 2 * B],
                                pf_ap(7),
                            )
                        else:
                            nc.vector.tensor_add(_ap(zbf_t[p]), _ap(zt_t[p]), pf_pair(p))
                    else:
                        # final step: fp32 state add inline, then ship it out
                        nc.vector.tensor_add(_ap(zt_t[p]), _ap(zt_t[p]), pf_pair(p))
                        nc.sync.dma_start(
                            zt_out[:, 2 * p * B : 2 * (p + 1) * B], _ap(zt_t[p])
                        )
            if i < NSTEP - 1:
                # fp32 state updates deferred past mm2 (the scheduler slots
                # them right behind each pair's zbf add)
                for p in range(4):
                    nc.vector.tensor_add(_ap(zt_t[p]), _ap(zt_t[p]), pf_pair(p))
                if not upd4:
                    # advance the coarse bias tile for step i+1
                    _BIAS_UPD_ENGINE(biast[:], biast[:], biasd[:])

    nc.compile()
    return nc


# --------------------------------------------------------------------------
# Variant A: fp32 batch-stationary (original baseline)
# --------------------------------------------------------------------------


def _build_program_a(mm_dtype=MM_DTYPE, repeat=1):
    import concourse.mybir as mybir
    from concourse import bacc
    from concourse.tile import TileContext

    nc = bacc.Bacc()
    f32 = mybir.dt.float32
    mmdt = getattr(mybir.dt, mm_dtype)
    TANH = mybir.ActivationFunctionType.Tanh

    zt_in = nc.dram_tensor("zt_in", [128, KD * B], mmdt, kind="ExternalInput")
    w1_d = nc.dram_tensor("w1", [128, KD * HID], mmdt, kind="ExternalInput")
    w2_d = nc.dram_tensor("w2", [128, KH * D], mmdt, kind="ExternalInput")
    biases_d = nc.dram_tensor("biases", [NSTEP, HID], mmdt, kind="ExternalInput")
    ident_d = nc.dram_tensor("ident", [128, 128], mmdt, kind="ExternalInput")
    ones_d = nc.dram_tensor("ones", [1, B], mmdt, kind="ExternalInput")
    zt_out = nc.dram_tensor("zt_out", [128, KD * B], mmdt, kind="ExternalOutput")

    with (
        TileContext(nc) as tc,
        tc.tile_pool(name="const", bufs=1) as cpool,
        tc.tile_pool(name="weights", bufs=1) as wpool,
        tc.tile_pool(name="state", bufs=1) as spool,
        tc.tile_pool(name="work", bufs=2) as hpool,
        tc.tile_pool(name="bias", bufs=2) as bpool,
        tc.tile_pool(name="psumh", bufs=2, space="PSUM") as ph_pool,
        tc.tile_pool(name="psumt", bufs=2, space="PSUM") as pt_pool,
        tc.tile_pool(name="psumf", bufs=2, space="PSUM") as pf_pool,
    ):
        ident_sb = cpool.tile([128, 128], mmdt, tag="ident")
        nc.sync.dma_start(ident_sb[:], ident_d[:])
        ones_sb = cpool.tile([1, B], mmdt, tag="ones")
        nc.sync.dma_start(ones_sb[:], ones_d[:])

        zt = spool.tile([128, KD * B], mmdt, tag="zt")
        nc.sync.dma_start(zt[:], zt_in[:])
        hT = spool.tile([128, KH * B], mmdt, tag="hT")

        w1t = []
        for k in range(KD):
            w = wpool.tile([128, HID], mmdt, tag=f"w1_{k}")
            nc.sync.dma_start(w[:], w1_d[:, k * HID : (k + 1) * HID])
            w1t.append(w)
        w2t = []
        for k in range(KH):
            w = wpool.tile([128, D], mmdt, tag=f"w2_{k}")
            nc.sync.dma_start(w[:], w2_d[:, k * D : (k + 1) * D])
            w2t.append(w)

        def scan_body(_iv=None):
            for i in range(NSTEP):
                bias_sb = bpool.tile([1, HID], mmdt, tag="bias")
                nc.sync.dma_start(bias_sb[:], biases_d[i : i + 1, :])

                phs = []
                for g in range(2):
                    ph = ph_pool.tile([128, 512], f32, tag="ph")
                    phs.append(ph)
                    for half in range(2):
                        c = 2 * g + half
                        nc.tensor.matmul(
                            ph[64 * half : 64 * half + 64, :],
                            ones_sb[:1, :],
                            bias_sb[:1, 512 * c : 512 * c + 512],
                            start=True,
                            stop=False,
                            tile_position=(0, 64 * half),
                        )
                    for k in range(KD):
                        for half in range(2):
                            c = 2 * g + half
                            nc.tensor.matmul(
                                ph[64 * half : 64 * half + 64, :],
                                zt[:, B * k : B * k + B],
                                w1t[k][:, 512 * c : 512 * c + 512],
                                start=False,
                                stop=(k == KD - 1),
                                tile_position=(0, 64 * half),
                            )

                for g in range(2):
                    h_bm = hpool.tile([128, 512], mmdt, tag="h_bm")
                    nc.scalar.activation(h_bm[:], phs[g][:], TANH)
                    pt = pt_pool.tile([128, 512], mmdt, tag="pt")
                    for u in range(4):
                        nc.tensor.matmul(
                            pt[:, 128 * u : 128 * u + 128],
                            h_bm[:, 128 * u : 128 * u + 128],
                            ident_sb[:],
                            is_transpose=True,
                            start=True,
                            stop=True,
                        )
                    nc.vector.tensor_copy(
                        hT[:, 512 * g : 512 * g + 512].rearrange(
                            "p (h u c) -> p h u c", h=2, u=4
                        ),
                        pt[:].rearrange("p (u h c) -> p h u c", u=4, h=2),
                    )

                pf = pf_pool.tile([128, 512], f32, tag="pf")
                for k in range(KH):
                    for half in range(2):
                        nc.tensor.matmul(
                            pf[64 * half : 64 * half + 64, :],
                            hT[:, B * k : B * k + B],
                            w2t[k][:, 512 * half : 512 * half + 512],
                            start=(k == 0),
                            stop=(k == KH - 1),
                            tile_position=(0, 64 * half),
                        )

                f_bm = hpool.tile([128, 512], mmdt, tag="f_bm")
                nc.vector.tensor_copy(f_bm[:], pf[:])
                pt2 = pt_pool.tile([128, 512], mmdt, tag="pt")
                for u in range(4):
                    nc.tensor.matmul(
                        pt2[:, 128 * u : 128 * u + 128],
                        f_bm[:, 128 * u : 128 * u + 128],
                        ident_sb[:],
                        is_transpose=True,
                        start=True,
                        stop=True,
                    )
                zt_v = zt[:].rearrange("p (h u c) -> p h u c", h=2, u=4)
                nc.vector.tensor_add(
                    zt_v, zt_v, pt2[:].rearrange("p (u h c) -> p h u c", u=4, h=2)
                )

        if repeat == 1:
            scan_body()
        else:
            with tc.For_i(0, repeat, 1) as _i:
                scan_body(_i)

        nc.sync.dma_start(zt_out[:], zt[:])

    nc.compile()
    return nc


# --------------------------------------------------------------------------
# Host-side packing
# --------------------------------------------------------------------------


def _pack_zT(shard):  # [B, D] -> [128, KD*B]
    return np.ascontiguousarray(
        shard.T.reshape(KD, 128, B).transpose(1, 0, 2).reshape(128, KD * B)
    )


def _unpack_zT(zt):  # [128, KD*B] -> [B, D]
    return zt.reshape(128, KD, B).transpose(1, 0, 2).reshape(D, B).T


def _host_common(z0, t, W1, b1, wt, W2, b2):
    t = np.asarray(t, F32)
    t0s, t1s = t[:-1], t[1:]
    h_seg = (t1s - t0s) / 2.0  # N_STEPS_PER_SEG = 2
    step_ts = (
        t0s[:, None] + h_seg[:, None] * np.arange(2, dtype=F32)[None, :]
    ).reshape(-1)
    step_hs = np.repeat(h_seg, 2)
    assert np.allclose(step_hs, step_hs[0]), "non-uniform Euler steps unsupported"
    scale = F32(step_hs[0])

    c = (scale * np.asarray(b2, F32)).astype(F32)  # [D]
    cW1 = (c.astype(np.float64) @ np.asarray(W1, np.float64)).astype(F32)  # [HID]
    biases = np.stack(
        [
            (np.asarray(b1, F32) + step_ts[i] * np.asarray(wt, F32) + i * cW1).astype(
                F32
            )
            for i in range(NSTEP)
        ]
    )  # [NSTEP, HID]
    return biases, scale, c


def _tile768(v):
    # [HID] -> [128, 768]: out[p, b*192 + r*64 + c] = v[(3b+r)*128 + p], r<3
    A = np.asarray(v, F32).reshape(KH, 128)[:12].reshape(4, 3, 128)
    return np.ascontiguousarray(
        np.broadcast_to(
            A.transpose(2, 0, 1)[:, :, :, None], (128, 4, 3, B)
        ).reshape(128, 768)
    ).astype(F32)


def _make_in_maps_b(z0, t, W1, b1, wt, W2, b2):
    import ml_dtypes

    bf16 = ml_dtypes.bfloat16
    z0 = np.asarray(z0, F32)
    biases, scale, c = _host_common(z0, t, W1, b1, wt, W2, b2)

    bias_cols = np.ascontiguousarray(
        biases.reshape(NSTEP, KH, 128).transpose(2, 0, 1).reshape(128, NSTEP * KH)
    )
    # coarse-path bias: step-0 values plus the (constant) per-step increment
    bias_base = _tile768(biases[0])
    bias_delta = _tile768(biases[1] - biases[0])
    # w1p[p, hm*KD*128 + k*128 + c] = W1[k*128+p, hm*128+c]
    w1p = np.ascontiguousarray(
        np.asarray(W1, F32)
        .reshape(KD, 128, KH, 128)
        .transpose(1, 2, 0, 3)
        .reshape(128, KD * HID)
    ).astype(bf16)
    # w2p[p, dm*KH*128 + q*128 + c] = W2'[q*128+p, dm*128+c]
    w2p = np.ascontiguousarray(
        (scale * np.asarray(W2, F32))
        .astype(F32)
        .reshape(KH, 128, KD, 128)
        .transpose(1, 2, 0, 3)
        .reshape(128, KH * D)
    ).astype(bf16)

    in_maps = []
    for core in range(NCORES):
        shard = z0[core * B : (core + 1) * B]
        ztp = _pack_zT(shard)
        in_maps.append(
            {
                "zt_in": ztp,
                "zbf_in": ztp.astype(bf16),
                "w1": w1p,
                "w2": w2p,
                "biases": bias_cols,
                "bias_base": bias_base,
                "bias_delta": bias_delta,
            }
        )
    return in_maps, c


def _make_in_maps_a(z0, t, W1, b1, wt, W2, b2):
    z0 = np.asarray(z0, F32)
    biases, scale, c = _host_common(z0, t, W1, b1, wt, W2, b2)
    w1p = np.ascontiguousarray(
        np.asarray(W1, F32)
        .reshape(KD, 128, HID)
        .transpose(1, 0, 2)
        .reshape(128, KD * HID)
    )
    w2p = np.ascontiguousarray(
        (scale * np.asarray(W2, F32))
        .astype(F32)
        .reshape(KH, 128, D)
        .transpose(1, 0, 2)
        .reshape(128, KH * D)
    )
    ident = np.eye(128, dtype=F32)
    ones = np.ones((1, B), F32)
    in_maps = []
    for core in range(NCORES):
        shard = z0[core * B : (core + 1) * B]
        in_maps.append(
            {
                "zt_in": _pack_zT(shard),
                "w1": w1p,
                "w2": w2p,
                "biases": biases,
                "ident": ident,
                "ones": ones,
            }
        )
    return in_maps, c


def run(z0, t, W1, b1, wt, W2, b2, trace=False, mm_dtype=MM_DTYPE, variant=VARIANT,
        **flags):
    from concourse.bass_utils import run_bass_kernel_spmd

    if variant == "b":
        in_maps, c = _make_in_maps_b(z0, t, W1, b1, wt, W2, b2)
        nc = _build_program_b(**flags)
    else:
        in_maps, c = _make_in_maps_a(z0, t, W1, b1, wt, W2, b2)
        nc = _build_program_a(mm_dtype=mm_dtype)
    res = run_bass_kernel_spmd(nc, in_maps, core_ids=list(range(NCORES)), trace=trace)

    outs = []
    for core in range(NCORES):
        z_shard = _unpack_zT(np.asarray(res.results[core]["zt_out"], F32))
        outs.append(z_shard)
    out = np.concatenate(outs, axis=0).astype(F32)
    out = out + (NSTEP * c)[None, :].astype(F32)
    return out.astype(F32), res


def kernel(z0, t, W1, b1, wt, W2, b2):
    out, _ = run(z0, t, W1, b1, wt, W2, b2, trace=False)
    return out
